# revision 14
# baseline (speedup 1.0000x reference)
"""Trainium2 Bass kernel for BatteryMoEFlattenIntraCycleMoELayer.

Computation (reference):
    gates = renorm(top2(softmax(logits) * mask))          # [B, E]
    x = cycle_curve_data.reshape(B, L, 900)
    out[b] = sum_e gates[b,e] * (x[b] @ W[e] + b[e])      # -> bf16 [B, L, 512]

Strategy:
  - Host: compute gates + top-2 routing (tiny), transpose x to feat-major
    [B, 901, 128] with a constant-1.0 row appended (folds the bias add into
    the matmul via weight augmentation W_aug = [W; b]).
  - Shard B across 8 cores (64 samples each). One SPMD program: routing is
    carried as *data* (per-sample W-slot element offsets, read into PE
    registers at runtime -> dynamic access patterns on the matmul moving
    operand), so the program does not depend on input values.
  - Device per sample: 2 experts x 8 K-chunks matmuls (N=512, float32r at
    full PE rate) accumulate x_aug @ W_aug[e] into 2 PSUM banks; ACT engine
    scales each by its gate (per-partition scalar AP from data); DVE adds
    and casts to bf16. The K=5 tail matmuls of both experts run packed in
    concurrent 32-row PE groups (host duplicates the tail rows at partition
    offset 32).
"""

import os
import sys

for _p in ("/opt/trn_rl_repo", "/root/.axon_site/_ro/trn_rl_repo"):
    if os.path.isdir(_p) and _p not in sys.path:
        sys.path.insert(0, _p)

import numpy as np
import ml_dtypes

import concourse.bass as bass
import concourse.mybir as mybir
import concourse.tile as tile
from concourse import bacc
from concourse.bass_utils import run_bass_kernel_spmd
from concourse.bass_values import RuntimeValue

B, L, CURVE_LEN = 512, 128, 300
FEAT = 3 * CURVE_LEN          # 900
FEAT_AUG = FEAT + 1           # 901 (bias row)
D_MODEL = 512
NUM_EXPERTS = 8
TOP_K = 2
EPS = 1e-9
N_CORES = 8
S = B // N_CORES              # 64 samples per core
N_KCH = 8                     # K chunks: 7 x 128 + 1 x 5
K_LAST = FEAT_AUG - 7 * 128   # 5

# matmul input dtype: float32r streams fp32 bits at full PE rate (N>=256)
MM_DT = mybir.dt.float32r

_CACHE = {}


def _build_nc():
    """Build the SPMD Bass program (routing-independent)."""
    nc = bacc.Bacc(trn_type="TRN2")
    f32 = mybir.dt.float32
    bf16 = mybir.dt.bfloat16
    i32 = mybir.dt.int32

    # x chunks 0..6: [S, 7*128, 128]; tail chunk (rows 896..900 + copy at
    # partition offset 32) as separate [S, 37, 128] tensor
    xt_h = nc.declare_dram_parameter("xt", [S, 7 * 128, L], MM_DT, isOutput=False)
    xtail_h = nc.declare_dram_parameter("xtail", [S, 37, L], MM_DT, isOutput=False)
    # w laid out per k-chunk: [k, part(<=128), expert, 512]
    w_h = nc.declare_dram_parameter("w", [N_KCH, 128, NUM_EXPERTS, D_MODEL], MM_DT,
                                    isOutput=False)
    g_h = nc.declare_dram_parameter("g", [128, 2 * S], f32, isOutput=False)
    widx_h = nc.declare_dram_parameter("widx", [1, 2 * S], i32, isOutput=False)
    y_h = nc.declare_dram_parameter("y", [S, L, D_MODEL], bf16, isOutput=True)

    with tile.TileContext(nc) as tc:
        with (
            tc.tile_pool(name="cpool", bufs=1) as cpool,
            tc.tile_pool(name="xpool", bufs=10) as xpool,
            tc.tile_pool(name="tpool", bufs=4) as tpool,
            tc.tile_pool(name="opool", bufs=3) as opool,
            tc.tile_pool(name="pspool", bufs=8, space="PSUM") as pspool,
        ):
            # --- constants: gates, routing offsets, weights ---
            g_sb = cpool.tile([128, 2 * S], f32)
            nc.sync.dma_start(out=g_sb[:, :], in_=g_h[:, :])
            widx_sb = cpool.tile([1, 2 * S], i32)
            nc.sync.dma_start(out=widx_sb[:, :], in_=widx_h[:, :])

            # W tiles (DMAs issued after the phase-1 x preloads below)
            w_sb = []
            for k in range(N_KCH):
                wt = cpool.tile([128, NUM_EXPERTS * D_MODEL], MM_DT,
                                name=f"w_sb_{k}")
                w_sb.append(wt)

            def load_w():
                # split each k-tile's DMA into 4 column chunks so the
                # transfers spread over many queues and k=0 lands early
                WCOL = NUM_EXPERTS * D_MODEL // 4   # 1024
                for k in range(N_KCH):
                    for c in range(4):
                        nc.sync.dma_start(
                            out=w_sb[k][:, c * WCOL: (c + 1) * WCOL],
                            in_=w_h[k, :, :, :].rearrange("p e d -> p (e d)")[
                                :, c * WCOL: (c + 1) * WCOL
                            ],
                        )

            # ring of PE registers for the per-sample W-slot offsets;
            # loaded in batches of 8 (4 samples) to amortize TENSOR_LOAD cost
            NRING = 16
            wregs = [nc.tensor.alloc_register(f"widx_reg{i}") for i in range(NRING)]
            WMAX = (NUM_EXPERTS - 1) * D_MODEL

            def load_x(s):
                x_sb = xpool.tile([128, N_KCH * 128], MM_DT, tag="x",
                                  name=f"x_sb_{s}")
                nc.sync.dma_start(
                    out=x_sb[:, : 4 * 128].rearrange("p (k l) -> p k l", k=4),
                    in_=xt_h[s, : 4 * 128, :].rearrange("(k p) l -> p k l", p=128),
                )
                nc.sync.dma_start(
                    out=x_sb[:, 4 * 128: 7 * 128].rearrange("p (k l) -> p k l", k=3),
                    in_=xt_h[s, 4 * 128: 7 * 128, :].rearrange(
                        "(k p) l -> p k l", p=128
                    ),
                )
                nc.sync.dma_start(
                    out=x_sb[:K_LAST, 7 * 128: 7 * 128 + 128],
                    in_=xtail_h[s, :K_LAST, :],
                )
                return x_sb

            def load_widx(s0):
                # 8 registers <- widx[2*s0 : 2*s0+8] (4 samples) in one load
                regs = [wregs[(2 * s0 + j) % NRING] for j in range(8)]
                nc.tensor.reg_load(regs, widx_sb[0:1, 2 * s0: 2 * s0 + 8])
                return [RuntimeValue(val=r, min_val=0, max_val=WMAX)
                        for r in regs]

            def mm_pair(ps, x_sb, rv, k, start, stop):
                kk = 128 if k < 7 else K_LAST
                nc.tensor.matmul(
                    ps[:, :], x_sb[:kk, k * 128: k * 128 + 128],
                    w_sb[k][:kk, bass.ds(rv, D_MODEL)],
                    start=start, stop=stop,
                )

            def combine(s, psA, psB):
                t1 = tpool.tile([128, D_MODEL], f32, tag="t", name=f"t1_{s}")
                t2 = tpool.tile([128, D_MODEL], f32, tag="t", name=f"t2_{s}")
                nc.scalar.mul(t1[:, :], psA[:, :], g_sb[:, 2 * s: 2 * s + 1])
                nc.scalar.mul(t2[:, :], psB[:, :], g_sb[:, 2 * s + 1: 2 * s + 2])
                o_sb = opool.tile([128, D_MODEL], bf16, tag="o", name=f"o_{s}")
                nc.vector.tensor_tensor(
                    o_sb[:, :], t1[:, :], t2[:, :], mybir.AluOpType.add
                )
                nc.sync.dma_start(out=y_h[s, :, :], in_=o_sb[:, :])

            # --- phase 1: first 8 samples, k-outer in 4-sample groups so the
            # PE starts as soon as w_sb[0] lands (W tiles stream in behind) ---
            PHASE1_GROUPS = 2
            # issue phase-1 x DMAs before the W DMAs so they aren't queued
            # behind the 16.8 MB weight load
            p1_xs = [load_x(s) for s in range(PHASE1_GROUPS * 4)]
            load_w()
            for grp in range(PHASE1_GROUPS):
                s0 = grp * 4
                xs = p1_xs[s0: s0 + 4]
                rvs = load_widx(s0)
                pss = [pspool.tile([128, D_MODEL], f32, tag="ps",
                                   name=f"ps_{s0}_{j}") for j in range(8)]
                for k in range(N_KCH):
                    for j in range(4):
                        mm_pair(pss[2 * j], xs[j], rvs[2 * j], k,
                                start=(k == 0), stop=(k == N_KCH - 1))
                        mm_pair(pss[2 * j + 1], xs[j], rvs[2 * j + 1], k,
                                start=(k == 0), stop=(k == N_KCH - 1))
                for j in range(4):
                    combine(s0 + j, pss[2 * j], pss[2 * j + 1])

            # --- phase 2: steady state, sample-major ---
            for s in range(PHASE1_GROUPS * 4, S):
                x_sb = load_x(s)
                if s % 4 == 0:
                    _rvs = load_widx(s)
                    rv_cache = _rvs
                rvA = rv_cache[2 * (s % 4)]
                rvB = rv_cache[2 * (s % 4) + 1]

                psA = pspool.tile([128, D_MODEL], f32, tag="ps",
                                  name=f"psA_{s}")
                psB = pspool.tile([128, D_MODEL], f32, tag="ps",
                                  name=f"psB_{s}")
                for k in range(N_KCH):
                    mm_pair(psA, x_sb, rvA, k, start=(k == 0),
                            stop=(k == N_KCH - 1))
                    mm_pair(psB, x_sb, rvB, k, start=(k == 0),
                            stop=(k == N_KCH - 1))
                combine(s, psA, psB)

    nc.finalize()  # Bacc: reg graph-coloring + codegen passes, then freeze
    return nc


def _gates_np(logits, moe_masks):
    """Mirror reference _gates in numpy (fp32)."""
    lg = logits.astype(np.float32)
    m = lg.max(axis=1, keepdims=True)
    e = np.exp(lg - m)
    g = e / e.sum(axis=1, keepdims=True)
    g = g * (moe_masks == 1).astype(np.float32)
    # top-2, ties -> lower index first (matches jax.lax.top_k)
    top_idx = np.argsort(-g, axis=1, kind="stable")[:, :TOP_K]
    rows = np.arange(g.shape[0])[:, None]
    gsel = g[rows, top_idx]                                  # [B, 2]
    gsel = gsel / (gsel.sum(axis=1, keepdims=True) + EPS)
    return gsel.astype(np.float32), top_idx.astype(np.int32)


def _prep_inputs(cycle_curve_data, logits, moe_masks, W, b):
    gsel, top_idx = _gates_np(logits, moe_masks)

    x = np.ascontiguousarray(
        cycle_curve_data.reshape(B, L, FEAT).transpose(0, 2, 1)
    ).astype(np.float32, copy=False)                         # [B, 900, 128]
    xt = x[:, : 7 * 128, :]                                  # [B, 896, 128]
    xtail = np.zeros((B, 37, L), np.float32)
    xtail[:, :4, :] = x[:, 7 * 128: FEAT, :]
    xtail[:, 4, :] = 1.0                                     # bias row
    xtail[:, 32:37, :] = xtail[:, :5, :]                     # row-group-1 copy

    w_aug = np.concatenate(
        [W.astype(np.float32), b.astype(np.float32)[:, None, :]], axis=1
    )                                                        # [E, 901, 512]
    w_host = np.zeros((N_KCH, 128, NUM_EXPERTS, D_MODEL), np.float32)
    for k in range(7):
        w_host[k] = w_aug[:, k * 128: (k + 1) * 128, :].transpose(1, 0, 2)
    w_host[7, :K_LAST] = w_aug[:, 7 * 128:, :].transpose(1, 0, 2)
    w_host[7, 32: 32 + K_LAST] = w_host[7, :K_LAST]          # row-group-1 copy

    in_maps = []
    for c in range(N_CORES):
        sl = slice(c * S, (c + 1) * S)
        g_rep = np.broadcast_to(
            gsel[sl].reshape(1, 2 * S), (128, 2 * S)
        ).copy()
        widx = (top_idx[sl].reshape(1, 2 * S) * D_MODEL).astype(np.int32)
        in_maps.append({
            "xt": np.ascontiguousarray(xt[sl]),
            "xtail": xtail[sl],
            "w": w_host,
            "g": g_rep,
            "widx": widx,
        })
    return in_maps


def kernel(cycle_curve_data, logits, moe_masks, W, b):
    if "nc" not in _CACHE:
        _CACHE["nc"] = _build_nc()
    nc = _CACHE["nc"]

    in_maps = _prep_inputs(cycle_curve_data, logits, moe_masks, W, b)

    trace = bool(int(os.environ.get("KERNEL_PROFILE", "0")))
    res = run_bass_kernel_spmd(
        nc, in_maps, core_ids=list(range(N_CORES)), trace=trace
    )
    _CACHE["last_results"] = res

    out = np.empty((B, L, D_MODEL), ml_dtypes.bfloat16)
    for c in range(N_CORES):
        out[c * S: (c + 1) * S] = res.results[c]["y"]
    return out


# revision 15
# speedup vs baseline: 1.0489x; 1.0489x over previous
"""Trainium2 Bass kernel for BatteryMoEFlattenIntraCycleMoELayer.

Computation (reference):
    gates = renorm(top2(softmax(logits) * mask))          # [B, E]
    x = cycle_curve_data.reshape(B, L, 900)
    out[b] = sum_e gates[b,e] * (x[b] @ W[e] + b[e])      # -> bf16 [B, L, 512]

Strategy:
  - Host: compute gates + top-2 routing (tiny), transpose x to feat-major
    [B, 901, 128] with a constant-1.0 row appended (folds the bias add into
    the matmul via weight augmentation W_aug = [W; b]).
  - Shard B across 8 cores (64 samples each). One SPMD program: routing is
    carried as *data* (per-sample W-slot element offsets, read into PE
    registers at runtime -> dynamic access patterns on the matmul moving
    operand), so the program does not depend on input values.
  - Device per sample: 2 experts x 8 K-chunks matmuls (N=512, float32r at
    full PE rate) accumulate x_aug @ W_aug[e] into 2 PSUM banks; ACT engine
    scales each by its gate (per-partition scalar AP from data); DVE adds
    and casts to bf16. The K=5 tail matmuls of both experts run packed in
    concurrent 32-row PE groups (host duplicates the tail rows at partition
    offset 32).
"""

import os
import sys

for _p in ("/opt/trn_rl_repo", "/root/.axon_site/_ro/trn_rl_repo"):
    if os.path.isdir(_p) and _p not in sys.path:
        sys.path.insert(0, _p)

import numpy as np
import ml_dtypes

import concourse.bass as bass
import concourse.mybir as mybir
import concourse.tile as tile
from concourse import bacc
from concourse.bass_utils import run_bass_kernel_spmd
from concourse.bass_values import RuntimeValue

B, L, CURVE_LEN = 512, 128, 300
FEAT = 3 * CURVE_LEN          # 900
FEAT_AUG = FEAT + 1           # 901 (bias row)
D_MODEL = 512
NUM_EXPERTS = 8
TOP_K = 2
EPS = 1e-9
N_CORES = 8
S = B // N_CORES              # 64 samples per core
N_KCH = 8                     # K chunks: 7 x 128 + 1 x 5
K_LAST = FEAT_AUG - 7 * 128   # 5

# matmul input dtype: float32r streams fp32 bits at full PE rate (N>=256)
MM_DT = mybir.dt.float32r

_CACHE = {}


def _build_nc():
    """Build the SPMD Bass program (routing-independent)."""
    nc = bacc.Bacc(trn_type="TRN2")
    f32 = mybir.dt.float32
    bf16 = mybir.dt.bfloat16
    i32 = mybir.dt.int32

    # x chunks 0..6: [S, 7*128, 128]; tail chunk (rows 896..900 + copy at
    # partition offset 32) as separate [S, 37, 128] tensor
    xt_h = nc.declare_dram_parameter("xt", [S, 128, 7 * 128], MM_DT, isOutput=False)
    xtail_h = nc.declare_dram_parameter("xtail", [S, K_LAST, L], MM_DT, isOutput=False)
    # w laid out per k-chunk: [k, part(<=128), expert, 512]
    w_h = nc.declare_dram_parameter("w", [N_KCH, 128, NUM_EXPERTS, D_MODEL], MM_DT,
                                    isOutput=False)
    g_h = nc.declare_dram_parameter("g", [128, 2 * S], f32, isOutput=False)
    widx_h = nc.declare_dram_parameter("widx", [1, 2 * S], i32, isOutput=False)
    y_h = nc.declare_dram_parameter("y", [S, L, D_MODEL], bf16, isOutput=True)

    with tile.TileContext(nc) as tc:
        with (
            tc.tile_pool(name="cpool", bufs=1) as cpool,
            tc.tile_pool(name="xpool", bufs=10) as xpool,
            tc.tile_pool(name="tpool", bufs=4) as tpool,
            tc.tile_pool(name="opool", bufs=3) as opool,
            tc.tile_pool(name="pspool", bufs=8, space="PSUM") as pspool,
        ):
            # --- constants: gates, routing offsets, weights ---
            g_sb = cpool.tile([128, 2 * S], f32)
            nc.sync.dma_start(out=g_sb[:, :], in_=g_h[:, :])
            widx_sb = cpool.tile([1, 2 * S], i32)
            nc.sync.dma_start(out=widx_sb[:, :], in_=widx_h[:, :])

            # W tiles (DMAs issued after the phase-1 x preloads below)
            w_sb = []
            for k in range(N_KCH):
                wt = cpool.tile([128, NUM_EXPERTS * D_MODEL], MM_DT,
                                name=f"w_sb_{k}")
                w_sb.append(wt)

            def load_w():
                # split each k-tile's DMA into 4 column chunks so the
                # transfers spread over many queues and k=0 lands early
                WCOL = NUM_EXPERTS * D_MODEL // 4   # 1024
                for k in range(N_KCH):
                    for c in range(4):
                        nc.sync.dma_start(
                            out=w_sb[k][:, c * WCOL: (c + 1) * WCOL],
                            in_=w_h[k, :, :, :].rearrange("p e d -> p (e d)")[
                                :, c * WCOL: (c + 1) * WCOL
                            ],
                        )

            # ring of PE registers for the per-sample W-slot offsets;
            # loaded in batches of 8 (4 samples) to amortize TENSOR_LOAD cost
            NRING = 16
            wregs = [nc.tensor.alloc_register(f"widx_reg{i}") for i in range(NRING)]
            WMAX = (NUM_EXPERTS - 1) * D_MODEL

            def load_x(s):
                # host layout is partition-major: per partition one fully
                # contiguous 7*128*4B run -> efficient single DMA
                x_sb = xpool.tile([128, N_KCH * 128], MM_DT, tag="x",
                                  name=f"x_sb_{s}")
                nc.sync.dma_start(
                    out=x_sb[:, : 7 * 128],
                    in_=xt_h[s, :, :],
                )
                nc.sync.dma_start(
                    out=x_sb[:K_LAST, 7 * 128: 7 * 128 + 128],
                    in_=xtail_h[s, :, :],
                )
                return x_sb

            def load_widx(s0):
                # 8 registers <- widx[2*s0 : 2*s0+8] (4 samples) in one load
                regs = [wregs[(2 * s0 + j) % NRING] for j in range(8)]
                nc.tensor.reg_load(regs, widx_sb[0:1, 2 * s0: 2 * s0 + 8])
                return [RuntimeValue(val=r, min_val=0, max_val=WMAX)
                        for r in regs]

            def mm_pair(ps, x_sb, rv, k, start, stop):
                kk = 128 if k < 7 else K_LAST
                nc.tensor.matmul(
                    ps[:, :], x_sb[:kk, k * 128: k * 128 + 128],
                    w_sb[k][:kk, bass.ds(rv, D_MODEL)],
                    start=start, stop=stop,
                )

            def combine(s, psA, psB):
                t1 = tpool.tile([128, D_MODEL], f32, tag="t", name=f"t1_{s}")
                t2 = tpool.tile([128, D_MODEL], f32, tag="t", name=f"t2_{s}")
                nc.scalar.mul(t1[:, :], psA[:, :], g_sb[:, 2 * s: 2 * s + 1])
                nc.scalar.mul(t2[:, :], psB[:, :], g_sb[:, 2 * s + 1: 2 * s + 2])
                o_sb = opool.tile([128, D_MODEL], bf16, tag="o", name=f"o_{s}")
                nc.vector.tensor_tensor(
                    o_sb[:, :], t1[:, :], t2[:, :], mybir.AluOpType.add
                )
                nc.sync.dma_start(out=y_h[s, :, :], in_=o_sb[:, :])

            # --- phase 1: first 8 samples, k-outer in 4-sample groups so the
            # PE starts as soon as w_sb[0] lands (W tiles stream in behind) ---
            PHASE1_GROUPS = 2
            # issue phase-1 x DMAs before the W DMAs so they aren't queued
            # behind the 16.8 MB weight load
            p1_xs = [load_x(s) for s in range(PHASE1_GROUPS * 4)]
            load_w()
            for grp in range(PHASE1_GROUPS):
                s0 = grp * 4
                xs = p1_xs[s0: s0 + 4]
                rvs = load_widx(s0)
                pss = [pspool.tile([128, D_MODEL], f32, tag="ps",
                                   name=f"ps_{s0}_{j}") for j in range(8)]
                for k in range(N_KCH):
                    for j in range(4):
                        mm_pair(pss[2 * j], xs[j], rvs[2 * j], k,
                                start=(k == 0), stop=(k == N_KCH - 1))
                        mm_pair(pss[2 * j + 1], xs[j], rvs[2 * j + 1], k,
                                start=(k == 0), stop=(k == N_KCH - 1))
                for j in range(4):
                    combine(s0 + j, pss[2 * j], pss[2 * j + 1])

            # --- phase 2: steady state, sample-major ---
            for s in range(PHASE1_GROUPS * 4, S):
                x_sb = load_x(s)
                if s % 4 == 0:
                    _rvs = load_widx(s)
                    rv_cache = _rvs
                rvA = rv_cache[2 * (s % 4)]
                rvB = rv_cache[2 * (s % 4) + 1]

                psA = pspool.tile([128, D_MODEL], f32, tag="ps",
                                  name=f"psA_{s}")
                psB = pspool.tile([128, D_MODEL], f32, tag="ps",
                                  name=f"psB_{s}")
                for k in range(N_KCH):
                    mm_pair(psA, x_sb, rvA, k, start=(k == 0),
                            stop=(k == N_KCH - 1))
                    mm_pair(psB, x_sb, rvB, k, start=(k == 0),
                            stop=(k == N_KCH - 1))
                combine(s, psA, psB)

    nc.finalize()  # Bacc: reg graph-coloring + codegen passes, then freeze
    return nc


def _gates_np(logits, moe_masks):
    """Mirror reference _gates in numpy (fp32)."""
    lg = logits.astype(np.float32)
    m = lg.max(axis=1, keepdims=True)
    e = np.exp(lg - m)
    g = e / e.sum(axis=1, keepdims=True)
    g = g * (moe_masks == 1).astype(np.float32)
    # top-2, ties -> lower index first (matches jax.lax.top_k)
    top_idx = np.argsort(-g, axis=1, kind="stable")[:, :TOP_K]
    rows = np.arange(g.shape[0])[:, None]
    gsel = g[rows, top_idx]                                  # [B, 2]
    gsel = gsel / (gsel.sum(axis=1, keepdims=True) + EPS)
    return gsel.astype(np.float32), top_idx.astype(np.int32)


def _prep_inputs(cycle_curve_data, logits, moe_masks, W, b):
    gsel, top_idx = _gates_np(logits, moe_masks)

    xf = cycle_curve_data.reshape(B, L, FEAT).astype(np.float32, copy=False)
    # xt[s, p, k, l] = x[s, l, k*128 + p]  -> [B, 128, 7*128]
    xt = np.ascontiguousarray(
        xf[:, :, : 7 * 128].reshape(B, L, 7, 128).transpose(0, 3, 2, 1)
    ).reshape(B, 128, 7 * 128)
    xtail = np.empty((B, K_LAST, L), np.float32)
    xtail[:, :4, :] = xf[:, :, 7 * 128: FEAT].transpose(0, 2, 1)
    xtail[:, 4, :] = 1.0                                     # bias row

    w_aug = np.concatenate(
        [W.astype(np.float32), b.astype(np.float32)[:, None, :]], axis=1
    )                                                        # [E, 901, 512]
    w_host = np.zeros((N_KCH, 128, NUM_EXPERTS, D_MODEL), np.float32)
    for k in range(7):
        w_host[k] = w_aug[:, k * 128: (k + 1) * 128, :].transpose(1, 0, 2)
    w_host[7, :K_LAST] = w_aug[:, 7 * 128:, :].transpose(1, 0, 2)
    w_host[7, 32: 32 + K_LAST] = w_host[7, :K_LAST]          # row-group-1 copy

    in_maps = []
    for c in range(N_CORES):
        sl = slice(c * S, (c + 1) * S)
        g_rep = np.broadcast_to(
            gsel[sl].reshape(1, 2 * S), (128, 2 * S)
        ).copy()
        widx = (top_idx[sl].reshape(1, 2 * S) * D_MODEL).astype(np.int32)
        in_maps.append({
            "xt": np.ascontiguousarray(xt[sl]),
            "xtail": xtail[sl],
            "w": w_host,
            "g": g_rep,
            "widx": widx,
        })
    return in_maps


def kernel(cycle_curve_data, logits, moe_masks, W, b):
    if "nc" not in _CACHE:
        _CACHE["nc"] = _build_nc()
    nc = _CACHE["nc"]

    in_maps = _prep_inputs(cycle_curve_data, logits, moe_masks, W, b)

    trace = bool(int(os.environ.get("KERNEL_PROFILE", "0")))
    res = run_bass_kernel_spmd(
        nc, in_maps, core_ids=list(range(N_CORES)), trace=trace
    )
    _CACHE["last_results"] = res

    out = np.empty((B, L, D_MODEL), ml_dtypes.bfloat16)
    for c in range(N_CORES):
        out[c * S: (c + 1) * S] = res.results[c]["y"]
    return out


# revision 16
# speedup vs baseline: 1.0566x; 1.0074x over previous
"""Trainium2 Bass kernel for BatteryMoEFlattenIntraCycleMoELayer.

Computation (reference):
    gates = renorm(top2(softmax(logits) * mask))          # [B, E]
    x = cycle_curve_data.reshape(B, L, 900)
    out[b] = sum_e gates[b,e] * (x[b] @ W[e] + b[e])      # -> bf16 [B, L, 512]

Strategy:
  - Host: compute gates + top-2 routing (tiny), transpose x to feat-major
    [B, 901, 128] with a constant-1.0 row appended (folds the bias add into
    the matmul via weight augmentation W_aug = [W; b]).
  - Shard B across 8 cores (64 samples each). One SPMD program: routing is
    carried as *data* (per-sample W-slot element offsets, read into PE
    registers at runtime -> dynamic access patterns on the matmul moving
    operand), so the program does not depend on input values.
  - Device per sample: 2 experts x 8 K-chunks matmuls (N=512, float32r at
    full PE rate) accumulate x_aug @ W_aug[e] into 2 PSUM banks; ACT engine
    scales each by its gate (per-partition scalar AP from data); DVE adds
    and casts to bf16. The K=5 tail matmuls of both experts run packed in
    concurrent 32-row PE groups (host duplicates the tail rows at partition
    offset 32).
"""

import os
import sys

for _p in ("/opt/trn_rl_repo", "/root/.axon_site/_ro/trn_rl_repo"):
    if os.path.isdir(_p) and _p not in sys.path:
        sys.path.insert(0, _p)

import numpy as np
import ml_dtypes

import concourse.bass as bass
import concourse.mybir as mybir
import concourse.tile as tile
from concourse import bacc
from concourse.bass_utils import run_bass_kernel_spmd
from concourse.bass_values import RuntimeValue

B, L, CURVE_LEN = 512, 128, 300
FEAT = 3 * CURVE_LEN          # 900
FEAT_AUG = FEAT + 1           # 901 (bias row)
D_MODEL = 512
NUM_EXPERTS = 8
TOP_K = 2
EPS = 1e-9
N_CORES = 8
S = B // N_CORES              # 64 samples per core
N_KCH = 8                     # K chunks: 7 x 128 + 1 x 5
K_LAST = FEAT_AUG - 7 * 128   # 5

# matmul input dtype: float32r streams fp32 bits at full PE rate (N>=256)
MM_DT = mybir.dt.float32r

_CACHE = {}


def _build_nc():
    """Build the SPMD Bass program (routing-independent)."""
    nc = bacc.Bacc(trn_type="TRN2")
    f32 = mybir.dt.float32
    bf16 = mybir.dt.bfloat16
    i32 = mybir.dt.int32

    # x chunks 0..6: [S, 7*128, 128]; tail chunk (rows 896..900 + copy at
    # partition offset 32) as separate [S, 37, 128] tensor
    xt_h = nc.declare_dram_parameter("xt", [S, 128, 7 * 128], MM_DT, isOutput=False)
    xtail_h = nc.declare_dram_parameter("xtail", [S, K_LAST, L], MM_DT, isOutput=False)
    # w laid out per k-chunk: [k, part(<=128), expert, 512]
    w_h = nc.declare_dram_parameter("w", [N_KCH, 128, NUM_EXPERTS, D_MODEL], MM_DT,
                                    isOutput=False)
    g_h = nc.declare_dram_parameter("g", [128, 2 * S], f32, isOutput=False)
    widx_h = nc.declare_dram_parameter("widx", [1, 2 * S], i32, isOutput=False)
    y_h = nc.declare_dram_parameter("y", [S, L, D_MODEL], bf16, isOutput=True)

    with tile.TileContext(nc) as tc:
        with (
            tc.tile_pool(name="cpool", bufs=1) as cpool,
            tc.tile_pool(name="xpool", bufs=10) as xpool,
            tc.tile_pool(name="tpool", bufs=4) as tpool,
            tc.tile_pool(name="opool", bufs=3) as opool,
            tc.tile_pool(name="pspool", bufs=8, space="PSUM") as pspool,
        ):
            # --- constants: gates, routing offsets, weights ---
            g_sb = cpool.tile([128, 2 * S], f32)
            nc.sync.dma_start(out=g_sb[:, :], in_=g_h[:, :])
            widx_sb = cpool.tile([1, 2 * S], i32)
            nc.sync.dma_start(out=widx_sb[:, :], in_=widx_h[:, :])

            # W tiles (DMAs issued after the phase-1 x preloads below)
            w_sb = []
            for k in range(N_KCH):
                wt = cpool.tile([128, NUM_EXPERTS * D_MODEL], MM_DT,
                                name=f"w_sb_{k}")
                w_sb.append(wt)

            def load_w(k):
                # split each k-tile's DMA into 4 column chunks so the
                # transfers spread over many queues
                WCOL = NUM_EXPERTS * D_MODEL // 4   # 1024
                for c in range(4):
                    nc.sync.dma_start(
                        out=w_sb[k][:, c * WCOL: (c + 1) * WCOL],
                        in_=w_h[k, :, :, :].rearrange("p e d -> p (e d)")[
                            :, c * WCOL: (c + 1) * WCOL
                        ],
                    )

            # ring of PE registers for the per-sample W-slot offsets;
            # loaded in batches of 8 (4 samples) to amortize TENSOR_LOAD cost
            NRING = 16
            wregs = [nc.tensor.alloc_register(f"widx_reg{i}") for i in range(NRING)]
            WMAX = (NUM_EXPERTS - 1) * D_MODEL

            def load_x(s):
                # host layout is partition-major: per partition one fully
                # contiguous 7*128*4B run -> efficient single DMA
                x_sb = xpool.tile([128, N_KCH * 128], MM_DT, tag="x",
                                  name=f"x_sb_{s}")
                nc.sync.dma_start(
                    out=x_sb[:, : 7 * 128],
                    in_=xt_h[s, :, :],
                )
                nc.sync.dma_start(
                    out=x_sb[:K_LAST, 7 * 128: 7 * 128 + 128],
                    in_=xtail_h[s, :, :],
                )
                return x_sb

            def load_widx(s0):
                # 8 registers <- widx[2*s0 : 2*s0+8] (4 samples) in one load
                regs = [wregs[(2 * s0 + j) % NRING] for j in range(8)]
                nc.tensor.reg_load(regs, widx_sb[0:1, 2 * s0: 2 * s0 + 8])
                return [RuntimeValue(val=r, min_val=0, max_val=WMAX)
                        for r in regs]

            def mm_pair(ps, x_sb, rv, k, start, stop):
                kk = 128 if k < 7 else K_LAST
                nc.tensor.matmul(
                    ps[:, :], x_sb[:kk, k * 128: k * 128 + 128],
                    w_sb[k][:kk, bass.ds(rv, D_MODEL)],
                    start=start, stop=stop,
                )

            def combine(s, psA, psB):
                t1 = tpool.tile([128, D_MODEL], f32, tag="t", name=f"t1_{s}")
                t2 = tpool.tile([128, D_MODEL], f32, tag="t", name=f"t2_{s}")
                nc.scalar.mul(t1[:, :], psA[:, :], g_sb[:, 2 * s: 2 * s + 1])
                nc.scalar.mul(t2[:, :], psB[:, :], g_sb[:, 2 * s + 1: 2 * s + 2])
                o_sb = opool.tile([128, D_MODEL], bf16, tag="o", name=f"o_{s}")
                nc.vector.tensor_tensor(
                    o_sb[:, :], t1[:, :], t2[:, :], mybir.AluOpType.add
                )
                nc.sync.dma_start(out=y_h[s, :, :], in_=o_sb[:, :])

            # --- phase 1: first 12 samples, k-outer in 4-sample groups so
            # the PE starts as soon as w_sb[0] lands; DMA issue order is
            # interleaved to match consumption order ---
            PHASE1_GROUPS = 3
            p1_xs = [load_x(s) for s in range(4)]
            load_w(0)
            p1_xs += [load_x(4), load_x(5)]
            load_w(1)
            p1_xs += [load_x(6), load_x(7)]
            load_w(2)
            p1_xs += [load_x(8), load_x(9)]
            load_w(3)
            p1_xs += [load_x(10), load_x(11)]
            for k in range(4, N_KCH):
                load_w(k)
            for grp in range(PHASE1_GROUPS):
                s0 = grp * 4
                xs = p1_xs[s0: s0 + 4]
                rvs = load_widx(s0)
                pss = [pspool.tile([128, D_MODEL], f32, tag="ps",
                                   name=f"ps_{s0}_{j}") for j in range(8)]
                for k in range(N_KCH):
                    for j in range(4):
                        mm_pair(pss[2 * j], xs[j], rvs[2 * j], k,
                                start=(k == 0), stop=(k == N_KCH - 1))
                        mm_pair(pss[2 * j + 1], xs[j], rvs[2 * j + 1], k,
                                start=(k == 0), stop=(k == N_KCH - 1))
                for j in range(4):
                    combine(s0 + j, pss[2 * j], pss[2 * j + 1])

            # --- phase 2: steady state, sample-major ---
            for s in range(PHASE1_GROUPS * 4, S):
                x_sb = load_x(s)
                if s % 4 == 0:
                    _rvs = load_widx(s)
                    rv_cache = _rvs
                rvA = rv_cache[2 * (s % 4)]
                rvB = rv_cache[2 * (s % 4) + 1]

                psA = pspool.tile([128, D_MODEL], f32, tag="ps",
                                  name=f"psA_{s}")
                psB = pspool.tile([128, D_MODEL], f32, tag="ps",
                                  name=f"psB_{s}")
                for k in range(N_KCH):
                    mm_pair(psA, x_sb, rvA, k, start=(k == 0),
                            stop=(k == N_KCH - 1))
                    mm_pair(psB, x_sb, rvB, k, start=(k == 0),
                            stop=(k == N_KCH - 1))
                combine(s, psA, psB)

    nc.finalize()  # Bacc: reg graph-coloring + codegen passes, then freeze
    return nc


def _gates_np(logits, moe_masks):
    """Mirror reference _gates in numpy (fp32)."""
    lg = logits.astype(np.float32)
    m = lg.max(axis=1, keepdims=True)
    e = np.exp(lg - m)
    g = e / e.sum(axis=1, keepdims=True)
    g = g * (moe_masks == 1).astype(np.float32)
    # top-2, ties -> lower index first (matches jax.lax.top_k)
    top_idx = np.argsort(-g, axis=1, kind="stable")[:, :TOP_K]
    rows = np.arange(g.shape[0])[:, None]
    gsel = g[rows, top_idx]                                  # [B, 2]
    gsel = gsel / (gsel.sum(axis=1, keepdims=True) + EPS)
    return gsel.astype(np.float32), top_idx.astype(np.int32)


def _prep_inputs(cycle_curve_data, logits, moe_masks, W, b):
    gsel, top_idx = _gates_np(logits, moe_masks)

    xf = cycle_curve_data.reshape(B, L, FEAT).astype(np.float32, copy=False)
    # xt[s, p, k, l] = x[s, l, k*128 + p]  -> [B, 128, 7*128]
    xt = np.ascontiguousarray(
        xf[:, :, : 7 * 128].reshape(B, L, 7, 128).transpose(0, 3, 2, 1)
    ).reshape(B, 128, 7 * 128)
    xtail = np.empty((B, K_LAST, L), np.float32)
    xtail[:, :4, :] = xf[:, :, 7 * 128: FEAT].transpose(0, 2, 1)
    xtail[:, 4, :] = 1.0                                     # bias row

    w_aug = np.concatenate(
        [W.astype(np.float32), b.astype(np.float32)[:, None, :]], axis=1
    )                                                        # [E, 901, 512]
    w_host = np.zeros((N_KCH, 128, NUM_EXPERTS, D_MODEL), np.float32)
    for k in range(7):
        w_host[k] = w_aug[:, k * 128: (k + 1) * 128, :].transpose(1, 0, 2)
    w_host[7, :K_LAST] = w_aug[:, 7 * 128:, :].transpose(1, 0, 2)
    w_host[7, 32: 32 + K_LAST] = w_host[7, :K_LAST]          # row-group-1 copy

    in_maps = []
    for c in range(N_CORES):
        sl = slice(c * S, (c + 1) * S)
        g_rep = np.broadcast_to(
            gsel[sl].reshape(1, 2 * S), (128, 2 * S)
        ).copy()
        widx = (top_idx[sl].reshape(1, 2 * S) * D_MODEL).astype(np.int32)
        in_maps.append({
            "xt": np.ascontiguousarray(xt[sl]),
            "xtail": xtail[sl],
            "w": w_host,
            "g": g_rep,
            "widx": widx,
        })
    return in_maps


def kernel(cycle_curve_data, logits, moe_masks, W, b):
    if "nc" not in _CACHE:
        _CACHE["nc"] = _build_nc()
    nc = _CACHE["nc"]

    in_maps = _prep_inputs(cycle_curve_data, logits, moe_masks, W, b)

    trace = bool(int(os.environ.get("KERNEL_PROFILE", "0")))
    res = run_bass_kernel_spmd(
        nc, in_maps, core_ids=list(range(N_CORES)), trace=trace
    )
    _CACHE["last_results"] = res

    out = np.empty((B, L, D_MODEL), ml_dtypes.bfloat16)
    for c in range(N_CORES):
        out[c * S: (c + 1) * S] = res.results[c]["y"]
    return out
